# revision 4
# baseline (speedup 1.0000x reference)
"""Trainium2 Bass kernel for nn_ConvchannelAttentionBlock.

reference (per batch b):
    S      = x @ x.T                      (C x C, symmetric; contraction over L)
    probs  = softmax(rowmax(S) - S)       == exp(rowmin(S) - S) / rowsum(...)
    read   = probs @ x                    (C x L)
    out    = eta * read + x

Sharding: data-parallel over B. Each of the 8 cores gets 4 batches and
runs an identical NEFF (SPMD) on its shard; outputs are concatenated.

Per-core pipeline (per batch), software-pipelined across batches:
  1. Host supplies x as bf16 [C, L] (residual + source of the eta-scaled
     fp8 copy) and x.T as fp8 [L, C] (pre-transposed and pre-cast on the
     host; partition p holds rows p*32..p*32+31 so every partition's DMA
     line is one contiguous 16 KB block — no DMA-transpose, no on-device
     input casts).
  2. mm1: full S = xT.T @ xT in fp8 DoubleRow (4 row panels x 16 MMs,
     N=512); per panel, rowmin on DVE and a PSUM->SBUF bf16 copy on ACT.
  3. Transposed softmax (S is symmetric):
       probsT[d, c] = exp(mn[c] - S[d, c]) / Z[c]
     The min-row is built with 4 tiny PE transposes + one rank-1 outer
     product (ones x mn-row) into PSUM; D = bias - S on DVE; U = exp(D)
     on ACT straight to fp8. No Ehat transposes at all.
     Z[c] = colsum(U) via a ones-vector matmul on PE; 1/Z via DVE
     reciprocal after 4 tiny PE transposes back to a column.
  4. mm2: R = U.T @ (eta*x) in fp8 DoubleRow (eta is folded into the
     rhs, computed on ACT/GpSimd from the bf16 x).
  5. Epilogue fused on DVE/GpSimd: out = (R * (1/Z)[row]) + x via
     scalar_tensor_tensor, bf16 out, 512 KB output DMAs on the scalar
     ring (loads ride the sync ring). Host concatenates + upcasts.
The tensor engine runs almost nothing but back-to-back matmuls; all
per-column softmax scales are folded into either the exp bias (shift
errors cancel in the softmax) or the per-row output scale.
"""

import sys

if "/opt/trn_rl_repo" not in sys.path:
    sys.path.insert(0, "/opt/trn_rl_repo")

import numpy as np
import ml_dtypes

import concourse.bacc as bacc
import concourse.tile as tile
from concourse import mybir

B, C, L = 32, 512, 4096
N_CORES = 8
NB = B // N_CORES  # batches per core
P = 128            # partitions
CM = C // P        # channel blocks (4)
KK = L // P        # l-slabs per partition (32)

_F32 = mybir.dt.float32
_BF16 = mybir.dt.bfloat16
_FP8 = mybir.dt.float8e4

DR = mybir.MatmulPerfMode.DoubleRow


def build_nc(nb=NB, dbg=False):
    """Build the per-core Bass kernel (nb batches of [C, L])."""
    nc = bacc.Bacc("TRN2", target_bir_lowering=False, debug=False)
    x_d = nc.dram_tensor("x", [nb, C, L], _BF16, kind="ExternalInput").ap()
    xt_d = nc.dram_tensor("xt", [nb, L, C], _FP8, kind="ExternalInput").ap()
    eta_d = nc.dram_tensor("eta128", [P, 1], _F32, kind="ExternalInput").ap()
    id_d = nc.dram_tensor("ident", [P, P], _BF16, kind="ExternalInput").ap()
    out_d = nc.dram_tensor("out", [nb, C, L], _BF16, kind="ExternalOutput").ap()
    if dbg:
        dbgU_d = nc.dram_tensor("dbgU", [P, CM * C], _FP8,
                                kind="ExternalOutput").ap()
        dbgS_d = nc.dram_tensor("dbgS", [P, CM * C], _BF16,
                                kind="ExternalOutput").ap()
        dbgZ_d = nc.dram_tensor("dbgZ", [P, CM], _F32,
                                kind="ExternalOutput").ap()

    with tile.TileContext(nc) as tc:
        with (
            tc.tile_pool(name="const", bufs=1) as const_pool,
            tc.tile_pool(name="x8", bufs=2) as x8_pool,
            tc.tile_pool(name="xt", bufs=2) as xt_pool,
            tc.tile_pool(name="xfe", bufs=2) as xfe_pool,
            tc.tile_pool(name="S", bufs=2) as s_pool,
            tc.tile_pool(name="U", bufs=2) as u_pool,
            tc.tile_pool(name="D", bufs=3) as d_pool,
            tc.tile_pool(name="wcol", bufs=2) as wcol_pool,
            tc.tile_pool(name="wrow", bufs=2) as wrow_pool,
            tc.tile_pool(name="zrow", bufs=2) as zrow_pool,
            tc.tile_pool(name="s8", bufs=2) as s8_pool,
            tc.tile_pool(name="stg", bufs=4) as st_pool,
            tc.tile_pool(name="pS", bufs=2, space="PSUM") as pS_pool,
            tc.tile_pool(name="pR", bufs=2, space="PSUM") as pR_pool,
            tc.tile_pool(name="pB", bufs=1, space="PSUM") as pB_pool,
            tc.tile_pool(name="pW", bufs=1, space="PSUM") as pW_pool,
            tc.tile_pool(name="pZ", bufs=1, space="PSUM") as pZ_pool,
            tc.tile_pool(name="pZC", bufs=1, space="PSUM") as pZC_pool,
        ):
            eta = const_pool.tile([P, 1], _F32, tag="eta")
            nc.sync.dma_start(eta[:], eta_d[:, :])
            ident = const_pool.tile([P, P], _BF16, tag="ident")
            nc.sync.dma_start(ident[:], id_d[:, :])
            ones_row = const_pool.tile([1, P], _BF16, tag="ones_row")
            nc.vector.memset(ones_row[:], 1.0)
            ones_col = const_pool.tile([P, 1], _FP8, tag="ones_col")
            nc.vector.memset(ones_col[:], 1.0)

            state = {}

            def emit_loads(b):
                xt = xt_pool.tile([P, KK * C], _FP8, tag="xt",
                                  name=f"xt_{b}")
                nc.sync.dma_start(
                    xt[:].rearrange("p (k q) -> p k q", k=KK),
                    xt_d[b].rearrange("(p k) q -> p k q", k=KK))
                x8 = x8_pool.tile([P, CM * L], _BF16, tag="x8",
                                  name=f"x8_{b}")
                nc.sync.dma_start(
                    x8[:].rearrange("p (s q) -> p s q", s=CM),
                    x_d[b].rearrange("(s p) q -> p s q", p=P))
                state[b] = {"x8": x8, "xt": xt}

            def emit_mm1(b):
                st = state[b]
                xv = st["xt"][:].rearrange("p (k q) -> p k q", k=KK)
                S = s_pool.tile([P, CM * C], _BF16, tag="S", name=f"S_{b}")
                wcol = wcol_pool.tile([P, CM], _BF16, tag="wcol",
                                      name=f"wcol_{b}")
                wrow_ps = pW_pool.tile([1, C], _F32, tag="pW")
                for m in range(CM):
                    ps = pS_pool.tile([P, C], _F32, tag="pS")
                    for g in range(KK // 2):
                        nc.tensor.matmul(
                            ps[:, :],
                            xv[:, 2 * g:2 * g + 2, m * P:(m + 1) * P],
                            xv[:, 2 * g:2 * g + 2, :],
                            start=(g == 0),
                            stop=(g == KK // 2 - 1),
                            perf_mode=DR,
                        )
                    nc.vector.tensor_reduce(
                        wcol[:, m:m + 1], ps[:, :],
                        axis=mybir.AxisListType.X, op=mybir.AluOpType.min)
                    nc.scalar.copy(S[:, m * C:(m + 1) * C], ps[:, :])
                    if m >= 1:
                        # transpose previous panel's min column into the row
                        nc.tensor.matmul(
                            wrow_ps[0:1, (m - 1) * P:m * P],
                            wcol[:, m - 1:m], ident[:],
                            start=True, stop=True)
                nc.tensor.matmul(
                    wrow_ps[0:1, (CM - 1) * P:CM * P],
                    wcol[:, CM - 1:CM], ident[:], start=True, stop=True)
                wrow = wrow_pool.tile([1, C], _BF16, tag="wrow",
                                      name=f"wrow_{b}")
                nc.vector.tensor_copy(wrow[0:1, :], wrow_ps[0:1, :])
                st["S"] = S
                st["wrow"] = wrow

            def emit_bias_d_exp(b):
                st = state[b]
                bias_ps = pB_pool.tile([P, C], _F32, tag="pB")
                nc.tensor.matmul(bias_ps[:, :], ones_row[0:1, :],
                                 st["wrow"][0:1, :], start=True, stop=True)
                U = u_pool.tile([P, CM * C], _FP8, tag="U", name=f"U_{b}")
                for j in range(CM):
                    Dt = d_pool.tile([P, C], _F32, tag="D")
                    nc.vector.tensor_tensor(
                        Dt[:], bias_ps[:], st["S"][:, j * C:(j + 1) * C],
                        op=mybir.AluOpType.subtract)
                    nc.scalar.activation(
                        U[:, j * C:(j + 1) * C], Dt[:],
                        mybir.ActivationFunctionType.Exp)
                st["U"] = U

            def emit_colsum(b):
                st = state[b]
                zrow_ps = pZ_pool.tile([1, C], _F32, tag="pZ")
                for j in range(CM):
                    nc.tensor.matmul(
                        zrow_ps[0:1, :], ones_col[:],
                        st["U"][:, j * C:(j + 1) * C],
                        start=(j == 0), stop=(j == CM - 1))
                zrow = zrow_pool.tile([1, C], _BF16, tag="zrow",
                                      name=f"zrow_{b}")
                nc.scalar.copy(zrow[0:1, :], zrow_ps[0:1, :])
                zc_ps = pZC_pool.tile([P, CM], _F32, tag="pZC")
                for m in range(CM):
                    nc.tensor.matmul(
                        zc_ps[:, m:m + 1], zrow[0:1, m * P:(m + 1) * P],
                        ident[0:1, 0:1], start=True, stop=True)
                s8 = s8_pool.tile([P, CM], _F32, tag="s8", name=f"s8_{b}")
                nc.vector.reciprocal(s8[:], zc_ps[:])
                st["s8"] = s8
                if dbg and b == 0:
                    nc.scalar.dma_start(dbgU_d[:, :], st["U"][:])
                    nc.scalar.dma_start(dbgS_d[:, :], st["S"][:])
                    nc.scalar.dma_start(dbgZ_d[:, :], s8[:])

            def emit_xfe(b):
                # eta * x cast to fp8 for mm2's rhs; 2 chunks on ACT,
                # 2 on GpSimd (DVE is the busiest elementwise engine).
                st = state[b]
                xfe = xfe_pool.tile([P, CM * L], _FP8, tag="xfe",
                                    name=f"xfe_{b}")
                nchk = CM * L // 4
                for i in range(4):
                    sl = slice(i * nchk, (i + 1) * nchk)
                    if i < 2:
                        nc.scalar.mul(xfe[:, sl], st["x8"][:, sl], eta[:])
                    else:
                        nc.gpsimd.tensor_scalar(
                            xfe[:, sl], st["x8"][:, sl], eta[:, 0:1], None,
                            op0=mybir.AluOpType.mult)
                st["xfe"] = xfe

            def emit_mm2(b, half):
                st = state[b]
                Uv = st["U"][:].rearrange("p (j q) -> p j q", j=CM)
                xfev = st["xfe"][:].rearrange("p (s q) -> p s q", s=CM)
                x8 = st["x8"]
                s8 = st["s8"]
                HW = 2048
                for m in (range(0, 2) if half == 0 else range(2, CM)):
                    for hh in range(L // HW):
                        stg = st_pool.tile([P, HW], _BF16, tag="stg",
                                           name=f"stg_{b}_{m}_{hh}")
                        for n2 in range(HW // 512):
                            n0 = hh * HW + n2 * 512
                            pr = pR_pool.tile([P, 512], _F32, tag="pR")
                            for g in range(CM // 2):
                                nc.tensor.matmul(
                                    pr[:, :],
                                    Uv[:, 2 * g:2 * g + 2,
                                       m * P:(m + 1) * P],
                                    xfev[:, 2 * g:2 * g + 2, n0:n0 + 512],
                                    start=(g == 0),
                                    stop=(g == CM // 2 - 1),
                                    perf_mode=DR,
                                )
                            # GpSimd has no PSUM port — epilogue must run
                            # on DVE (reads pr from PSUM).
                            nc.vector.scalar_tensor_tensor(
                                stg[:, n2 * 512:(n2 + 1) * 512],
                                pr[:], s8[:, m:m + 1],
                                x8[:, m * L + n0:m * L + n0 + 512],
                                op0=mybir.AluOpType.mult,
                                op1=mybir.AluOpType.add)
                        nc.scalar.dma_start(
                            out_d[b, m * P:(m + 1) * P,
                                  hh * HW:(hh + 1) * HW], stg[:])
                if half == 1:
                    st.clear()

            emit_loads(0)
            if nb > 1:
                emit_loads(1)
            for b in range(nb):
                emit_mm1(b)
                if b + 2 < nb:
                    emit_loads(b + 2)
                if b >= 1:
                    emit_mm2(b - 1, 0)
                emit_bias_d_exp(b)
                if b >= 1:
                    emit_mm2(b - 1, 1)
                emit_colsum(b)
                emit_xfe(b)
            emit_mm2(nb - 1, 0)
            emit_mm2(nb - 1, 1)
    nc.compile()
    return nc


_NC_CACHE = {}


def _get_nc(dbg=False):
    if dbg not in _NC_CACHE:
        _NC_CACHE[dbg] = build_nc(dbg=dbg)
    return _NC_CACHE[dbg]


def prepare_in_maps(minibatch: np.ndarray, eta: np.ndarray):
    eta128 = np.ascontiguousarray(
        np.broadcast_to(eta.reshape(1, 1).astype(np.float32), (P, 1)))
    ident = np.eye(P, dtype=ml_dtypes.bfloat16)
    in_maps = []
    for i in range(N_CORES):
        xc = minibatch[i * NB:(i + 1) * NB]
        in_maps.append({
            "x": np.ascontiguousarray(xc.astype(ml_dtypes.bfloat16)),
            "xt": np.ascontiguousarray(
                xc.transpose(0, 2, 1)).astype(ml_dtypes.float8_e4m3),
            "eta128": eta128,
            "ident": ident,
        })
    return in_maps


def collect_out(res):
    out = np.concatenate([res.results[i]["out"] for i in range(N_CORES)],
                         axis=0)
    return out.astype(np.float32)


def kernel(minibatch: np.ndarray, eta: np.ndarray) -> np.ndarray:
    from concourse.bass_utils import run_bass_kernel_spmd

    assert minibatch.shape == (B, C, L)
    nc = _get_nc()
    in_maps = prepare_in_maps(minibatch, eta)
    res = run_bass_kernel_spmd(nc, in_maps, core_ids=list(range(N_CORES)))
    return collect_out(res)
